# revision 1
# baseline (speedup 1.0000x reference)
"""Trainium2 Bass kernel for nn_Conv1d_fft (B=16, Cin=Cout=128, L=4096, K=129, PAD=32).

The reference computes the conv via FFT with circular length 4160, output
truncated to 4032. Because 4160 >= L + 2*PAD and only the first 4032 samples
are kept, the circular wrap only ever touches zero padding, so the whole op
is exactly a plain cross-correlation (PyTorch-style Conv1d with padding=32)
plus a bias:

    out[b, o, n] = bias[o] + sum_{i, t} weight[o, i, t] * xp[b, i, n + t]

with xp = x zero-padded by 32 on each side (length 4160), n in [0, 4032).

Strategy: data-parallel over batch (2 batches per core, 8 cores). Per core,
the conv is 129 shifted matmuls accumulated in PSUM per output tile:
lhsT = weight[:, :, t] transposed to (Cin, Cout), rhs = xp window (Cin, 504).
Weight is pre-transposed on the host to (Cin, K*Cout) so DMA is contiguous.

Matmul dtype: float16 (default). fp16 carries 10 mantissa bits — the same
precision as TF32/f32r — while streaming at the full 16-bit rate with fast
(FWL) weight loads. Our data fits fp16's 5-bit exponent easily (x ~ N(0,1),
w in [-0.008, 0.008]; PSUM accumulation is always fp32), so f16 gives
f32r accuracy at bf16 speed.

Measured on trn2 (8 cores, NTFF profile): 458.6 us, rel err 2.85e-4 vs
the fp32 FFT reference. Breakdown: ~10.6 us DMA startup (prioritized
first-tile slices), 2064 matmuls at 214.3 ns each — exactly the PE stream
roofline for N=504 (504 cycles @ 2.4 GHz + NX dispatch), zero PE gaps —
and ~11.6 us Tile kernel-tail barrier. Alternatives measured: plain fp32
1894 us (7.5e-7); f32r 493.5 us (2.85e-4; streams at ~1.08 cyc/row on
silicon — confirmed by eliminating 87% of LDWEIGHTS via walrus
--enable-ldw-opt=true + tap-outer weight sharing with no spacing change);
bf16 473 us (2.1e-3, no accuracy benefit over f16 at the same speed).
"""

import os
import numpy as np

import concourse.bass as bass
import concourse.bacc as bacc
import concourse.tile as tile
import concourse.mybir as mybir
from concourse.bass_utils import run_bass_kernel_spmd

B, CIN, COUT, L, K = 16, 128, 128, 4096, 129
PAD = 32
OUT_LEN = 2 * PAD + L - (K - 1)  # 4032
LP = L + 2 * PAD                 # 4160
N_CORES = 8
BPC = B // N_CORES               # batches per core
TW = 504                         # output tile width (8 * 504 = 4032)
NT = OUT_LEN // TW
WCHUNKS = 8                      # weight DMA split (taps per chunk below)

F32 = mybir.dt.float32
BF16 = mybir.dt.bfloat16
F16 = mybir.dt.float16
F32R = mybir.dt.float32r

_cache = {}


def _tap_chunks():
    """Contiguous tap chunks for the weight DMA split. The first chunk is
    tiny so the very first matmuls wait on a ~0.26 MB transfer instead of
    ~1 MB; later chunks are bigger for DMA efficiency."""
    sizes = [4, 14, 18, 18, 19, 19, 19, 18]
    assert sum(sizes) == K and len(sizes) == WCHUNKS
    bounds = [0]
    for s in sizes:
        bounds.append(bounds[-1] + s)
    return [(bounds[i], bounds[i + 1]) for i in range(WCHUNKS)]


def _build_program(mode: str, order: str):
    """mode: f32 | f32r | bf16;  order: tap_inner | tap_outer."""
    io_dt = {"f32": F32, "f32r": F32R, "bf16": BF16, "f16": F16}[mode]
    nc = bacc.Bacc("TRN2", target_bir_lowering=False, debug=False,
                   num_devices=N_CORES)

    x_d = nc.dram_tensor("x", [BPC, CIN, LP], io_dt, kind="ExternalInput").ap()
    w_d = nc.dram_tensor("w", [CIN, K * COUT], io_dt, kind="ExternalInput").ap()
    b_d = nc.dram_tensor("b", [COUT, 1], F32, kind="ExternalInput").ap()
    o_d = nc.dram_tensor("out", [BPC, COUT, OUT_LEN], F32,
                         kind="ExternalOutput").ap()

    chunks = _tap_chunks()

    with tile.TileContext(nc) as tc:
        with (
            tc.tile_pool(name="wp", bufs=1) as wp,
            tc.tile_pool(name="xp", bufs=1) as xp,
            tc.tile_pool(name="bp", bufs=1) as bp,
            tc.tile_pool(name="op", bufs=4) as op,
            tc.tile_pool(name="ps", bufs=8 if order == "tap_inner" else 1,
                         space=bass.MemorySpace.PSUM) as ps,
        ):
            # DMA priority order: the first matmul group (batch 0, tile 0)
            # only needs x[0][:, :1136] and weight chunk 0, so issue those
            # first; the rest streams in behind while the PE is already busy.
            w_sb = [wp.tile([CIN, (t1 - t0) * COUT], io_dt, tag=f"w{ci}",
                            name=f"wsb{ci}")
                    for ci, (t0, t1) in enumerate(chunks)]
            x_sb = [xp.tile([CIN, LP], io_dt, tag=f"x{b}", name=f"xsb{b}")
                    for b in range(BPC)]

            # Critical path: tile (b=0, j=0) reads x[0][:, :632] and all taps.
            # Issue a small x slice, then weight chunks (smallest first), with
            # the rest of x interleaved behind the first two weight chunks.
            XA = TW + COUT      # columns needed by the first tile group
            XB = 2 * TW + COUT  # ... by the first two tile groups
            nc.sync.dma_start(x_sb[0][:, :XA], x_d[0][:, :XA])
            for ci, (t0, t1) in enumerate(chunks):
                nc.sync.dma_start(w_sb[ci][:], w_d[:, t0 * COUT:t1 * COUT])
                if ci == 0:
                    nc.sync.dma_start(x_sb[0][:, XA:XB], x_d[0][:, XA:XB])
                elif ci == 1:
                    nc.sync.dma_start(x_sb[0][:, XB:], x_d[0][:, XB:])
            b_sb = bp.tile([COUT, 1], F32)
            nc.sync.dma_start(b_sb[:], b_d[:])
            for b in range(1, BPC):
                nc.sync.dma_start(x_sb[b][:], x_d[b])

            def w_ap(t):
                for ci, (t0, t1) in enumerate(chunks):
                    if t0 <= t < t1:
                        return w_sb[ci][:, (t - t0) * COUT:(t - t0 + 1) * COUT]
                raise AssertionError

            def drain(psum_tile, b, j):
                o_sb = op.tile([COUT, TW], F32)
                nc.vector.tensor_scalar_add(o_sb[:], psum_tile[:], b_sb[:])
                nc.sync.dma_start(o_d[b][:, j * TW:(j + 1) * TW], o_sb[:])

            if order == "tap_inner":
                for b in range(BPC):
                    for j in range(NT):
                        acc = ps.tile([COUT, TW], F32)
                        for t in range(K):
                            nc.tensor.matmul(
                                acc[:],
                                w_ap(t),
                                x_sb[b][:, j * TW + t: j * TW + t + TW],
                                start=(t == 0), stop=(t == K - 1),
                            )
                        drain(acc, b, j)
            else:  # tap_outer
                for b in range(BPC):
                    accs = [ps.tile([COUT, TW], F32, tag=f"acc{j}", name=f"accs{j}")
                            for j in range(NT)]
                    for t in range(K):
                        for j in range(NT):
                            nc.tensor.matmul(
                                accs[j][:],
                                w_ap(t),
                                x_sb[b][:, j * TW + t: j * TW + t + TW],
                                start=(t == 0), stop=(t == K - 1),
                            )
                    for j in range(NT):
                        drain(accs[j], b, j)

    nc.compile()
    return nc


def _get_program(mode, order):
    key = (mode, order)
    if key not in _cache:
        _cache[key] = _build_program(mode, order)
    return _cache[key]


def _round_tf32(a: np.ndarray) -> np.ndarray:
    """Round fp32 to TF32 (10 mantissa bits), round-to-nearest-even."""
    u = np.ascontiguousarray(a, dtype=np.float32).view(np.uint32)
    r = (u + np.uint32(0xFFF) + ((u >> np.uint32(13)) & np.uint32(1))) \
        & np.uint32(0xFFFFE000)
    return r.view(np.float32)


def kernel(x, weight, bias, _trace=False, _trace_kwargs=None):
    mode = os.environ.get("BASS_CONV_MODE", "f16")
    order = os.environ.get("BASS_CONV_ORDER", "tap_inner")
    nc = _get_program(mode, order)

    if mode == "bf16":
        import ml_dtypes
        io_np = ml_dtypes.bfloat16
    elif mode == "f16":
        io_np = np.float16
    else:
        io_np = np.float32

    xp = np.zeros((B, CIN, LP), dtype=np.float32)
    xp[:, :, PAD:PAD + L] = x
    # (Cout, Cin, K) -> (Cin, K, Cout) so per-tap lhsT slices are contiguous
    wT = np.ascontiguousarray(np.transpose(
        np.asarray(weight, dtype=np.float32), (1, 2, 0)))
    if mode == "f32r":
        xp = _round_tf32(xp)
        wT = _round_tf32(wT)
    xp = np.ascontiguousarray(xp.astype(io_np))
    wT = np.ascontiguousarray(wT.astype(io_np)).reshape(CIN, K * COUT)
    b2 = np.ascontiguousarray(np.asarray(bias, np.float32).reshape(COUT, 1))

    in_maps = [
        {"x": xp[c * BPC:(c + 1) * BPC], "w": wT, "b": b2}
        for c in range(N_CORES)
    ]
    res = run_bass_kernel_spmd(
        nc, in_maps, list(range(N_CORES)),
        trace=_trace, **(_trace_kwargs or {}),
    )
    out = np.concatenate([res.results[c]["out"] for c in range(N_CORES)],
                         axis=0).astype(np.float32)
    if _trace:
        return out, res
    return out



# revision 6
# speedup vs baseline: 2.1410x; 2.1410x over previous
"""Trainium2 Bass kernel for nn_Conv1d_fft (B=16, Cin=Cout=128, L=4096, K=129, PAD=32).

The reference computes the conv via FFT with circular length 4160, output
truncated to 4032. Because 4160 >= L + 2*PAD and only the first 4032 samples
are kept, the circular wrap only ever touches zero padding, so the whole op
is exactly a plain cross-correlation (PyTorch-style Conv1d with padding=32)
plus a bias:

    out[b, o, n] = bias[o] + sum_{i, t} weight[o, i, t] * xp[b, i, n + t]

with xp = x zero-padded by 32 on each side (length 4160), n in [0, 4032).

Strategy: data-parallel over batch (2 batches per core, 8 cores). Per core,
the conv is 129 shifted matmuls accumulated in PSUM per output tile:
lhsT = weight[:, :, t] transposed to (Cin, Cout), rhs = xp window (Cin, 504).
Weight is pre-transposed on the host to (Cin, K*Cout) so DMA is contiguous.

Matmul dtype: float16 (default). fp16 carries 10 mantissa bits — the same
precision as TF32/f32r — while streaming at the full 16-bit rate with fast
(FWL) weight loads. Our data fits fp16's 5-bit exponent easily (x ~ N(0,1),
w in [-0.008, 0.008]; PSUM accumulation is always fp32), so f16 gives
f32r accuracy at bf16 speed.

Measured on trn2 (8 cores, NTFF profile): 458.6 us, rel err 2.85e-4 vs
the fp32 FFT reference. Breakdown: ~10.6 us DMA startup (prioritized
first-tile slices), 2064 matmuls at 214.3 ns each — exactly the PE stream
roofline for N=504 (504 cycles @ 2.4 GHz + NX dispatch), zero PE gaps —
and ~11.6 us Tile kernel-tail barrier. Alternatives measured: plain fp32
1894 us (7.5e-7); f32r 493.5 us (2.85e-4; streams at ~1.08 cyc/row on
silicon — confirmed by eliminating 87% of LDWEIGHTS via walrus
--enable-ldw-opt=true + tap-outer weight sharing with no spacing change);
bf16 473 us (2.1e-3, no accuracy benefit over f16 at the same speed).
"""

import os
import numpy as np

import concourse.bass as bass
import concourse.bacc as bacc
import concourse.tile as tile
import concourse.mybir as mybir
from concourse.bass_utils import run_bass_kernel_spmd

B, CIN, COUT, L, K = 16, 128, 128, 4096, 129
PAD = 32
OUT_LEN = 2 * PAD + L - (K - 1)  # 4032
LP = L + 2 * PAD                 # 4160
N_CORES = 8
BPC = B // N_CORES               # batches per core
TW = 504                         # output tile width (8 * 504 = 4032)
NT = OUT_LEN // TW
WCHUNKS = 8                      # weight DMA split (taps per chunk below)

F32 = mybir.dt.float32
BF16 = mybir.dt.bfloat16
F16 = mybir.dt.float16
F32R = mybir.dt.float32r

_cache = {}


def _tap_chunks():
    """Contiguous tap chunks for the weight DMA split. The first chunk is
    tiny so the very first matmuls wait on a ~0.26 MB transfer instead of
    ~1 MB; later chunks are bigger for DMA efficiency."""
    sizes = [4, 14, 18, 18, 19, 19, 19, 18]
    assert sum(sizes) == K and len(sizes) == WCHUNKS
    bounds = [0]
    for s in sizes:
        bounds.append(bounds[-1] + s)
    return [(bounds[i], bounds[i + 1]) for i in range(WCHUNKS)]


F8 = mybir.dt.float8e4
SX = 8.0          # host scale on x before e4m3 quantization
SW = 8192.0       # host scale on w before e4m3 quantization
INV_S = 1.0 / (SX * SW)

# tap ranges for the f8 weight DMA split: all even-sized (DoubleRow pairs
# never straddle a chunk), plus a final 1-tap chunk for tap 128.
F8_CHUNKS = [(0, 4), (4, 18), (18, 36), (36, 54), (54, 74),
             (74, 92), (92, 110), (110, 128), (128, 129)]


def _build_f8_program():
    """fp8e4m3 DoubleRow: each matmul contracts 2 taps at 2 rows/cycle.

    rhs pair = (x window at tap 2p, x window at tap 2p+1); the +1 shift is a
    second SBUF copy of x offset by one sample. lhsT pair = the two taps'
    (Cin, Cout) slices. 64 DoubleRow matmuls + 1 plain fp8 matmul (tap 128)
    per output tile. PSUM accumulates fp32; drain = psum * 1/(SX*SW) + bias
    fused on the scalar engine.
    """
    nc = bacc.Bacc("TRN2", target_bir_lowering=False, debug=False,
                   num_devices=N_CORES)
    x_d = nc.dram_tensor("x", [BPC, CIN, LP], F8, kind="ExternalInput").ap()
    w_d = nc.dram_tensor("w", [CIN, K, COUT], F8, kind="ExternalInput").ap()
    b_d = nc.dram_tensor("b", [COUT, 1], F32, kind="ExternalInput").ap()
    o_d = nc.dram_tensor("out", [BPC, COUT, OUT_LEN], F32,
                         kind="ExternalOutput").ap()

    with tile.TileContext(nc) as tc:
        with (
            tc.tile_pool(name="wp", bufs=1) as wp,
            tc.tile_pool(name="xp", bufs=1) as xp,
            tc.tile_pool(name="bp", bufs=1) as bp,
            tc.tile_pool(name="op", bufs=4) as op,
            tc.tile_pool(name="ps", bufs=8, space=bass.MemorySpace.PSUM) as ps,
        ):
            w_sb = [wp.tile([CIN, t1 - t0, COUT], F8, tag=f"w{ci}",
                            name=f"wsb{ci}")
                    for ci, (t0, t1) in enumerate(F8_CHUNKS)]
            x_sb = [xp.tile([CIN, 2, LP], F8, tag=f"x{b}", name=f"xsb{b}")
                    for b in range(BPC)]

            XA = TW + COUT      # columns needed by the first tile group
            XB = 2 * TW + COUT
            nc.sync.dma_start(x_sb[0][:, 0, :XA], x_d[0][:, :XA])
            nc.sync.dma_start(x_sb[0][:, 1, :XA], x_d[0][:, 1:XA + 1])
            for ci, (t0, t1) in enumerate(F8_CHUNKS):
                nc.sync.dma_start(w_sb[ci][:], w_d[:, t0:t1, :])
                if ci == 0:
                    nc.sync.dma_start(x_sb[0][:, 0, XA:XB], x_d[0][:, XA:XB])
                    nc.sync.dma_start(x_sb[0][:, 1, XA:XB],
                                      x_d[0][:, XA + 1:XB + 1])
                elif ci == 1:
                    nc.sync.dma_start(x_sb[0][:, 0, XB:], x_d[0][:, XB:])
                    nc.sync.dma_start(x_sb[0][:, 1, XB:LP - 1],
                                      x_d[0][:, XB + 1:])
            b_sb = bp.tile([COUT, 1], F32)
            nc.sync.dma_start(b_sb[:], b_d[:])
            for b in range(1, BPC):
                nc.sync.dma_start(x_sb[b][:, 0, :], x_d[b])
                nc.sync.dma_start(x_sb[b][:, 1, :LP - 1], x_d[b][:, 1:])

            def w_pair(p):
                t = 2 * p
                for ci, (t0, t1) in enumerate(F8_CHUNKS):
                    if t0 <= t < t1:
                        return w_sb[ci][:, t - t0:t - t0 + 2, :]
                raise AssertionError

            for b in range(BPC):
                for j in range(NT):
                    acc = ps.tile([COUT, TW], F32)
                    for p in range(K // 2):
                        nc.tensor.matmul(
                            acc[:],
                            w_pair(p),
                            x_sb[b][:, :, j * TW + 2 * p:j * TW + 2 * p + TW],
                            start=(p == 0), stop=False,
                            perf_mode=mybir.MatmulPerfMode.DoubleRow,
                        )
                    nc.tensor.matmul(
                        acc[:],
                        w_sb[-1][:, 0, :],
                        x_sb[b][:, 0, j * TW + K - 1:j * TW + K - 1 + TW],
                        start=False, stop=True,
                    )
                    o_sb = op.tile([COUT, TW], F32)
                    nc.scalar.activation(
                        o_sb[:], acc[:], mybir.ActivationFunctionType.Identity,
                        bias=b_sb[:], scale=INV_S)
                    nc.sync.dma_start(o_d[b][:, j * TW:(j + 1) * TW], o_sb[:])

    nc.compile()
    return nc


# ---------------------------------------------------------------------------
# FFA (fast FIR / polyphase Karatsuba) mode.
#
# out[n] = sum_t w[t] xp[n+t] splits by even/odd phases of n and t into 4
# quarter-size correlations; Karatsuba computes them with 3:
#   P0 = corr(X0, W0)        W0[s] = w[2s]
#   Q  = corr([0]+X1, W1h)   W1h[s] = w[2s-1] (leading zero)
#   PS = corr(X0+X1, W0+W1h)
#   out[2m] = P0[m] + Q[m];  out[2m+1] = PS[m] - P0[m] - Q[m+1]
# Applied recursively FFA_LVL times: 3^L correlations of (K>>L)+1 taps on
# 2^L-decimated signals -> (4/3)^L fewer PE MACs at full f16 precision.
# Signals stay phase-separated end to end: the host ships x de-interleaved
# into 2^L phases (layout only); the S-combos, recombination tree and the
# final interleave run on the vector/scalar engines under the matmuls.
# ---------------------------------------------------------------------------
FFA_LVL = int(os.environ.get("BASS_FFA_LVL", "4"))
NPH = 2 ** FFA_LVL
NLEAF = 3 ** FFA_LVL
LEAF_TAPS = (K >> FFA_LVL) + 1
PH_LEN = LP // NPH
LMARG = 8
PH_PAD = LMARG + PH_LEN + 24
RC = OUT_LEN // NPH              # valid cols per root output comp
LEAF_N = RC + FFA_LVL            # leaf matmul width (extra cols guard shifts)
CTW = LEAF_N + 4                 # out-comp tile width
WBUFS = 6


def _build_ffa_program():
    nc = bacc.Bacc("TRN2", target_bir_lowering=False, debug=False,
                   num_devices=N_CORES)
    x_d = nc.dram_tensor("x", [BPC, NPH, CIN, PH_PAD], F16,
                         kind="ExternalInput").ap()
    w_d = nc.dram_tensor("w", [NLEAF, CIN, LEAF_TAPS, COUT], F16,
                         kind="ExternalInput").ap()
    b_d = nc.dram_tensor("b", [COUT, 1], F32, kind="ExternalInput").ap()
    o_d = nc.dram_tensor("out", [BPC, COUT, OUT_LEN], F32,
                         kind="ExternalOutput").ap()

    with tile.TileContext(nc) as tc:
        with (
            tc.tile_pool(name="xp", bufs=1) as xpool,
            tc.tile_pool(name="wp", bufs=1) as wpool,
            tc.tile_pool(name="bp", bufs=1) as bpool,
            tc.tile_pool(name="oc", bufs=1) as ocpool,
            tc.tile_pool(name="of", bufs=3) as ofpool,
            tc.tile_pool(name="ps", bufs=8, space=bass.MemorySpace.PSUM) as ps,
        ):
            b_sb = bpool.tile([COUT, 1], F32)
            nc.sync.dma_start(b_sb[:], b_d[:])

            # --- X phases + S-combo tree (per batch) ---------------------
            x_ph = {}
            for b in range(BPC):
                for p in range(NPH):
                    t = xpool.tile([CIN, PH_PAD], F16, tag=f"ph{b}_{p}",
                                   name=f"ph{b}_{p}")
                    nc.sync.dma_start(t[:], x_d[b, p])
                    x_ph[(b, p)] = t

            def x_children(comps, b, lvl, path):
                """comps: list of (tile, base). Returns (c0, c1, cS)."""
                r = len(comps)
                c0 = comps[0::2]
                last_t, last_b = comps[-1]
                c1 = [(last_t, last_b - 1)] + comps[1:-1:2]
                codd = comps[1::2]
                cS = []
                for i in range(r // 2):
                    t0, o0 = c0[i]
                    t1, o1 = codd[i]
                    st = xpool.tile([CIN, PH_PAD], F16,
                                    tag=f"xs{b}_{path}_{i}",
                                    name=f"xs{b}_{path}_{i}")
                    wdt = PH_PAD - LMARG
                    nc.vector.tensor_add(
                        st[:, 4:4 + wdt],
                        t0[:, o0 - LMARG + 4:o0 - LMARG + 4 + wdt],
                        t1[:, o1 - LMARG + 4:o1 - LMARG + 4 + wdt])
                    cS.append((st, LMARG))
                return c0, c1, cS

            def build_x_leaves(b):
                leaves = []

                def rec(comps, lvl, path):
                    if lvl == FFA_LVL:
                        assert len(comps) == 1
                        leaves.append(comps[0])
                        return
                    c0, c1, cS = x_children(comps, b, lvl, path)
                    rec(c0, lvl + 1, path + "0")
                    rec(c1, lvl + 1, path + "1")
                    rec(cS, lvl + 1, path + "2")

                rec([(x_ph[(b, p)], LMARG) for p in range(NPH)], 0, "")
                return leaves

            x_leaves = {b: build_x_leaves(b) for b in range(BPC)}

            # --- output DFS: leaf matmuls + recombination tree -----------
            leaf_ctr = [0]

            def emit(lvl, slot):
                wdt = RC + lvl
                if lvl == FFA_LVL:
                    lc = leaf_ctr[0]
                    leaf_ctr[0] += 1
                    w_sb = wpool.tile([CIN, LEAF_TAPS, COUT], F16,
                                      tag=f"w{lc % WBUFS}",
                                      name=f"w{lc % WBUFS}")
                    nc.sync.dma_start(w_sb[:], w_d[lc])
                    res = {}
                    for b in range(BPC):
                        xt, base = x_leaves[b][lc]
                        acc = ps.tile([COUT, LEAF_N], F32)
                        for s in range(LEAF_TAPS):
                            nc.tensor.matmul(
                                acc[:], w_sb[:, s, :],
                                xt[:, base + s:base + s + LEAF_N],
                                start=(s == 0), stop=(s == LEAF_TAPS - 1))
                        d = ocpool.tile([COUT, CTW], F16,
                                        tag=f"oc{b}_{lvl}_{slot}_0",
                                        name=f"oc{b}_{lvl}_{slot}_0")
                        nc.scalar.activation(
                            d[:, :LEAF_N], acc[:],
                            mybir.ActivationFunctionType.Copy)
                        res[b] = [d]
                    return res

                ch0 = emit(lvl + 1, 0)
                ch1 = emit(lvl + 1, 1)
                chS = emit(lvl + 1, 2)
                r2 = 2 ** (FFA_LVL - lvl - 1)
                res = {}
                for b in range(BPC):
                    comps = []
                    for pp in range(r2):
                        et = ocpool.tile([COUT, CTW], F16,
                                         tag=f"oc{b}_{lvl}_{slot}_e{pp}",
                                         name=f"oc{b}_{lvl}_{slot}_e{pp}")
                        nc.vector.tensor_add(et[:, :wdt],
                                             ch0[b][pp][:, :wdt],
                                             ch1[b][pp][:, :wdt])
                        ot = ocpool.tile([COUT, CTW], F16,
                                         tag=f"oc{b}_{lvl}_{slot}_o{pp}",
                                         name=f"oc{b}_{lvl}_{slot}_o{pp}")
                        nc.vector.tensor_sub(ot[:, :wdt],
                                             chS[b][pp][:, :wdt],
                                             ch0[b][pp][:, :wdt])
                        if pp + 1 < r2:
                            qs = ch1[b][pp + 1][:, 0:wdt]
                        else:
                            qs = ch1[b][0][:, 1:wdt + 1]
                        nc.vector.tensor_sub(ot[:, :wdt], ot[:, :wdt], qs)
                        comps.append(et)
                        comps.append(ot)
                    res[b] = comps
                return res

            root = emit(0, 0)

            # --- final interleave + bias -> fp32, DMA out ----------------
            HC = OUT_LEN // 2            # 2016
            HR = RC // 2                 # 126 cols per comp per half
            for b in range(BPC):
                for h in range(2):
                    o32 = ofpool.tile([COUT, HC], F32)
                    for p in range(NPH):
                        nc.scalar.activation(
                            o32[:, p:p + (HR - 1) * NPH + 1:NPH],
                            root[b][p][:, h * HR:(h + 1) * HR],
                            mybir.ActivationFunctionType.Identity,
                            bias=b_sb[:], scale=1.0)
                    nc.sync.dma_start(o_d[b][:, h * HC:(h + 1) * HC], o32[:])

    nc.compile()
    return nc


def _build_program(mode: str, order: str):
    """mode: f32 | f32r | bf16 | f16 | f8 | ffa;  order: tap_inner | tap_outer."""
    if mode == "f8":
        return _build_f8_program()
    if mode == "ffa":
        return _build_ffa_program()
    io_dt = {"f32": F32, "f32r": F32R, "bf16": BF16, "f16": F16}[mode]
    nc = bacc.Bacc("TRN2", target_bir_lowering=False, debug=False,
                   num_devices=N_CORES)

    x_d = nc.dram_tensor("x", [BPC, CIN, LP], io_dt, kind="ExternalInput").ap()
    w_d = nc.dram_tensor("w", [CIN, K * COUT], io_dt, kind="ExternalInput").ap()
    b_d = nc.dram_tensor("b", [COUT, 1], F32, kind="ExternalInput").ap()
    o_d = nc.dram_tensor("out", [BPC, COUT, OUT_LEN], F32,
                         kind="ExternalOutput").ap()

    chunks = _tap_chunks()

    with tile.TileContext(nc) as tc:
        with (
            tc.tile_pool(name="wp", bufs=1) as wp,
            tc.tile_pool(name="xp", bufs=1) as xp,
            tc.tile_pool(name="bp", bufs=1) as bp,
            tc.tile_pool(name="op", bufs=4) as op,
            tc.tile_pool(name="ps", bufs=8 if order == "tap_inner" else 1,
                         space=bass.MemorySpace.PSUM) as ps,
        ):
            # DMA priority order: the first matmul group (batch 0, tile 0)
            # only needs x[0][:, :1136] and weight chunk 0, so issue those
            # first; the rest streams in behind while the PE is already busy.
            w_sb = [wp.tile([CIN, (t1 - t0) * COUT], io_dt, tag=f"w{ci}",
                            name=f"wsb{ci}")
                    for ci, (t0, t1) in enumerate(chunks)]
            x_sb = [xp.tile([CIN, LP], io_dt, tag=f"x{b}", name=f"xsb{b}")
                    for b in range(BPC)]

            # Critical path: tile (b=0, j=0) reads x[0][:, :632] and all taps.
            # Issue a small x slice, then weight chunks (smallest first), with
            # the rest of x interleaved behind the first two weight chunks.
            XA = TW + COUT      # columns needed by the first tile group
            XB = 2 * TW + COUT  # ... by the first two tile groups
            nc.sync.dma_start(x_sb[0][:, :XA], x_d[0][:, :XA])
            for ci, (t0, t1) in enumerate(chunks):
                nc.sync.dma_start(w_sb[ci][:], w_d[:, t0 * COUT:t1 * COUT])
                if ci == 0:
                    nc.sync.dma_start(x_sb[0][:, XA:XB], x_d[0][:, XA:XB])
                elif ci == 1:
                    nc.sync.dma_start(x_sb[0][:, XB:], x_d[0][:, XB:])
            b_sb = bp.tile([COUT, 1], F32)
            nc.sync.dma_start(b_sb[:], b_d[:])
            for b in range(1, BPC):
                nc.sync.dma_start(x_sb[b][:], x_d[b])

            def w_ap(t):
                for ci, (t0, t1) in enumerate(chunks):
                    if t0 <= t < t1:
                        return w_sb[ci][:, (t - t0) * COUT:(t - t0 + 1) * COUT]
                raise AssertionError

            def drain(psum_tile, b, j):
                o_sb = op.tile([COUT, TW], F32)
                nc.vector.tensor_scalar_add(o_sb[:], psum_tile[:], b_sb[:])
                nc.sync.dma_start(o_d[b][:, j * TW:(j + 1) * TW], o_sb[:])

            if order == "tap_inner":
                for b in range(BPC):
                    for j in range(NT):
                        acc = ps.tile([COUT, TW], F32)
                        for t in range(K):
                            nc.tensor.matmul(
                                acc[:],
                                w_ap(t),
                                x_sb[b][:, j * TW + t: j * TW + t + TW],
                                start=(t == 0), stop=(t == K - 1),
                            )
                        drain(acc, b, j)
            else:  # tap_outer
                for b in range(BPC):
                    accs = [ps.tile([COUT, TW], F32, tag=f"acc{j}", name=f"accs{j}")
                            for j in range(NT)]
                    for t in range(K):
                        for j in range(NT):
                            nc.tensor.matmul(
                                accs[j][:],
                                w_ap(t),
                                x_sb[b][:, j * TW + t: j * TW + t + TW],
                                start=(t == 0), stop=(t == K - 1),
                            )
                    for j in range(NT):
                        drain(accs[j], b, j)

    nc.compile()
    return nc


def _get_program(mode, order):
    key = (mode, order)
    if key not in _cache:
        _cache[key] = _build_program(mode, order)
    return _cache[key]


def _round_tf32(a: np.ndarray) -> np.ndarray:
    """Round fp32 to TF32 (10 mantissa bits), round-to-nearest-even."""
    u = np.ascontiguousarray(a, dtype=np.float32).view(np.uint32)
    r = (u + np.uint32(0xFFF) + ((u >> np.uint32(13)) & np.uint32(1))) \
        & np.uint32(0xFFFFE000)
    return r.view(np.float32)


def _ffa_w_leaves(weight):
    """Leaf filters of the FFA tree: (NLEAF, CIN, LEAF_TAPS, COUT) f16."""
    wt = np.ascontiguousarray(np.transpose(
        np.asarray(weight, np.float32), (2, 1, 0)))   # (K, CIN, COUT)
    nodes = [wt]
    for _ in range(FFA_LVL):
        nxt = []
        for a in nodes:
            z = np.zeros_like(a[:1])
            w0 = a[0::2]
            w1h = np.concatenate([z, a[1::2]], axis=0)
            nxt += [w0, w1h, w0 + w1h]
        nodes = nxt
    arr = np.stack(nodes)                              # (NLEAF, T, CIN, COUT)
    return np.ascontiguousarray(
        arr.transpose(0, 2, 1, 3).astype(np.float16))


def _ffa_x_phases(xp):
    """De-interleave xp into 2^L phases with zero margins (layout only)."""
    ph = np.zeros((B, NPH, CIN, PH_PAD), np.float16)
    ph[:, :, :, LMARG:LMARG + PH_LEN] = \
        xp.reshape(B, CIN, PH_LEN, NPH).transpose(0, 3, 1, 2)
    return np.ascontiguousarray(ph)


def kernel(x, weight, bias, _trace=False, _trace_kwargs=None):
    mode = os.environ.get("BASS_CONV_MODE", "ffa")
    order = os.environ.get("BASS_CONV_ORDER", "tap_inner")
    nc = _get_program(mode, order)

    if mode == "ffa":
        xp = np.zeros((B, CIN, LP), dtype=np.float32)
        xp[:, :, PAD:PAD + L] = x
        xph = _ffa_x_phases(xp)
        wl = _ffa_w_leaves(weight)
        b2 = np.ascontiguousarray(np.asarray(bias, np.float32)
                                  .reshape(COUT, 1))
        in_maps = [
            {"x": xph[c * BPC:(c + 1) * BPC], "w": wl, "b": b2}
            for c in range(N_CORES)
        ]
        res = run_bass_kernel_spmd(
            nc, in_maps, list(range(N_CORES)),
            trace=_trace, **(_trace_kwargs or {}),
        )
        out = np.concatenate(
            [res.results[c]["out"] for c in range(N_CORES)],
            axis=0).astype(np.float32)
        if _trace:
            return out, res
        return out

    if mode == "bf16":
        import ml_dtypes
        io_np = ml_dtypes.bfloat16
    elif mode == "f16":
        io_np = np.float16
    elif mode == "f8":
        import ml_dtypes
        io_np = ml_dtypes.float8_e4m3
    else:
        io_np = np.float32

    xp = np.zeros((B, CIN, LP), dtype=np.float32)
    xp[:, :, PAD:PAD + L] = x
    # (Cout, Cin, K) -> (Cin, K, Cout) so per-tap lhsT slices are contiguous
    wT = np.ascontiguousarray(np.transpose(
        np.asarray(weight, dtype=np.float32), (1, 2, 0)))
    if mode == "f32r":
        xp = _round_tf32(xp)
        wT = _round_tf32(wT)
    if mode == "f8":
        xp = np.clip(xp * SX, -240.0, 240.0)
        wT = np.clip(wT * SW, -240.0, 240.0)
    xp = np.ascontiguousarray(xp.astype(io_np))
    wT = np.ascontiguousarray(wT.astype(io_np))
    if mode != "f8":
        wT = wT.reshape(CIN, K * COUT)
    b2 = np.ascontiguousarray(np.asarray(bias, np.float32).reshape(COUT, 1))

    in_maps = [
        {"x": xp[c * BPC:(c + 1) * BPC], "w": wT, "b": b2}
        for c in range(N_CORES)
    ]
    res = run_bass_kernel_spmd(
        nc, in_maps, list(range(N_CORES)),
        trace=_trace, **(_trace_kwargs or {}),
    )
    out = np.concatenate([res.results[c]["out"] for c in range(N_CORES)],
                         axis=0).astype(np.float32)
    if _trace:
        return out, res
    return out



# revision 12
# speedup vs baseline: 2.1521x; 1.0052x over previous
"""Trainium2 Bass kernel for nn_Conv1d_fft (B=16, Cin=Cout=128, L=4096, K=129, PAD=32).

The reference computes the conv via FFT with circular length 4160, output
truncated to 4032. Because 4160 >= L + 2*PAD and only the first 4032 samples
are kept, the circular wrap only ever touches zero padding, so the whole op
is exactly a plain cross-correlation (PyTorch-style Conv1d with padding=32)
plus a bias:

    out[b, o, n] = bias[o] + sum_{i, t} weight[o, i, t] * xp[b, i, n + t]

with xp = x zero-padded by 32 on each side (length 4160), n in [0, 4032).

Strategy: data-parallel over batch (2 batches per core, 8 cores). Per core,
the conv is 129 shifted matmuls accumulated in PSUM per output tile:
lhsT = weight[:, :, t] transposed to (Cin, Cout), rhs = xp window (Cin, 504).
Weight is pre-transposed on the host to (Cin, K*Cout) so DMA is contiguous.

Matmul dtype: float16 (default). fp16 carries 10 mantissa bits — the same
precision as TF32/f32r — while streaming at the full 16-bit rate with fast
(FWL) weight loads. Our data fits fp16's 5-bit exponent easily (x ~ N(0,1),
w in [-0.008, 0.008]; PSUM accumulation is always fp32), so f16 gives
f32r accuracy at bf16 speed.

Measured on trn2 (8 cores, NTFF profile): 458.6 us, rel err 2.85e-4 vs
the fp32 FFT reference. Breakdown: ~10.6 us DMA startup (prioritized
first-tile slices), 2064 matmuls at 214.3 ns each — exactly the PE stream
roofline for N=504 (504 cycles @ 2.4 GHz + NX dispatch), zero PE gaps —
and ~11.6 us Tile kernel-tail barrier. Alternatives measured: plain fp32
1894 us (7.5e-7); f32r 493.5 us (2.85e-4; streams at ~1.08 cyc/row on
silicon — confirmed by eliminating 87% of LDWEIGHTS via walrus
--enable-ldw-opt=true + tap-outer weight sharing with no spacing change);
bf16 473 us (2.1e-3, no accuracy benefit over f16 at the same speed).
"""

import os
import numpy as np

import concourse.bass as bass
import concourse.bacc as bacc
import concourse.tile as tile
import concourse.mybir as mybir
from concourse.bass_utils import run_bass_kernel_spmd

B, CIN, COUT, L, K = 16, 128, 128, 4096, 129
PAD = 32
OUT_LEN = 2 * PAD + L - (K - 1)  # 4032
LP = L + 2 * PAD                 # 4160
N_CORES = 8
BPC = B // N_CORES               # batches per core
TW = 504                         # output tile width (8 * 504 = 4032)
NT = OUT_LEN // TW
WCHUNKS = 8                      # weight DMA split (taps per chunk below)

F32 = mybir.dt.float32
BF16 = mybir.dt.bfloat16
F16 = mybir.dt.float16
F32R = mybir.dt.float32r

_cache = {}


def _tap_chunks():
    """Contiguous tap chunks for the weight DMA split. The first chunk is
    tiny so the very first matmuls wait on a ~0.26 MB transfer instead of
    ~1 MB; later chunks are bigger for DMA efficiency."""
    sizes = [4, 14, 18, 18, 19, 19, 19, 18]
    assert sum(sizes) == K and len(sizes) == WCHUNKS
    bounds = [0]
    for s in sizes:
        bounds.append(bounds[-1] + s)
    return [(bounds[i], bounds[i + 1]) for i in range(WCHUNKS)]


F8 = mybir.dt.float8e4
SX = 8.0          # host scale on x before e4m3 quantization
SW = 8192.0       # host scale on w before e4m3 quantization
INV_S = 1.0 / (SX * SW)

# tap ranges for the f8 weight DMA split: all even-sized (DoubleRow pairs
# never straddle a chunk), plus a final 1-tap chunk for tap 128.
F8_CHUNKS = [(0, 4), (4, 18), (18, 36), (36, 54), (54, 74),
             (74, 92), (92, 110), (110, 128), (128, 129)]


def _build_f8_program():
    """fp8e4m3 DoubleRow: each matmul contracts 2 taps at 2 rows/cycle.

    rhs pair = (x window at tap 2p, x window at tap 2p+1); the +1 shift is a
    second SBUF copy of x offset by one sample. lhsT pair = the two taps'
    (Cin, Cout) slices. 64 DoubleRow matmuls + 1 plain fp8 matmul (tap 128)
    per output tile. PSUM accumulates fp32; drain = psum * 1/(SX*SW) + bias
    fused on the scalar engine.
    """
    nc = bacc.Bacc("TRN2", target_bir_lowering=False, debug=False,
                   num_devices=N_CORES)
    x_d = nc.dram_tensor("x", [BPC, CIN, LP], F8, kind="ExternalInput").ap()
    w_d = nc.dram_tensor("w", [CIN, K, COUT], F8, kind="ExternalInput").ap()
    b_d = nc.dram_tensor("b", [COUT, 1], F32, kind="ExternalInput").ap()
    o_d = nc.dram_tensor("out", [BPC, COUT, OUT_LEN], F32,
                         kind="ExternalOutput").ap()

    with tile.TileContext(nc) as tc:
        with (
            tc.tile_pool(name="wp", bufs=1) as wp,
            tc.tile_pool(name="xp", bufs=1) as xp,
            tc.tile_pool(name="bp", bufs=1) as bp,
            tc.tile_pool(name="op", bufs=4) as op,
            tc.tile_pool(name="ps", bufs=8, space=bass.MemorySpace.PSUM) as ps,
        ):
            w_sb = [wp.tile([CIN, t1 - t0, COUT], F8, tag=f"w{ci}",
                            name=f"wsb{ci}")
                    for ci, (t0, t1) in enumerate(F8_CHUNKS)]
            x_sb = [xp.tile([CIN, 2, LP], F8, tag=f"x{b}", name=f"xsb{b}")
                    for b in range(BPC)]

            XA = TW + COUT      # columns needed by the first tile group
            XB = 2 * TW + COUT
            nc.sync.dma_start(x_sb[0][:, 0, :XA], x_d[0][:, :XA])
            nc.sync.dma_start(x_sb[0][:, 1, :XA], x_d[0][:, 1:XA + 1])
            for ci, (t0, t1) in enumerate(F8_CHUNKS):
                nc.sync.dma_start(w_sb[ci][:], w_d[:, t0:t1, :])
                if ci == 0:
                    nc.sync.dma_start(x_sb[0][:, 0, XA:XB], x_d[0][:, XA:XB])
                    nc.sync.dma_start(x_sb[0][:, 1, XA:XB],
                                      x_d[0][:, XA + 1:XB + 1])
                elif ci == 1:
                    nc.sync.dma_start(x_sb[0][:, 0, XB:], x_d[0][:, XB:])
                    nc.sync.dma_start(x_sb[0][:, 1, XB:LP - 1],
                                      x_d[0][:, XB + 1:])
            b_sb = bp.tile([COUT, 1], F32)
            nc.sync.dma_start(b_sb[:], b_d[:])
            for b in range(1, BPC):
                nc.sync.dma_start(x_sb[b][:, 0, :], x_d[b])
                nc.sync.dma_start(x_sb[b][:, 1, :LP - 1], x_d[b][:, 1:])

            def w_pair(p):
                t = 2 * p
                for ci, (t0, t1) in enumerate(F8_CHUNKS):
                    if t0 <= t < t1:
                        return w_sb[ci][:, t - t0:t - t0 + 2, :]
                raise AssertionError

            for b in range(BPC):
                for j in range(NT):
                    acc = ps.tile([COUT, TW], F32)
                    for p in range(K // 2):
                        nc.tensor.matmul(
                            acc[:],
                            w_pair(p),
                            x_sb[b][:, :, j * TW + 2 * p:j * TW + 2 * p + TW],
                            start=(p == 0), stop=False,
                            perf_mode=mybir.MatmulPerfMode.DoubleRow,
                        )
                    nc.tensor.matmul(
                        acc[:],
                        w_sb[-1][:, 0, :],
                        x_sb[b][:, 0, j * TW + K - 1:j * TW + K - 1 + TW],
                        start=False, stop=True,
                    )
                    o_sb = op.tile([COUT, TW], F32)
                    nc.scalar.activation(
                        o_sb[:], acc[:], mybir.ActivationFunctionType.Identity,
                        bias=b_sb[:], scale=INV_S)
                    nc.sync.dma_start(o_d[b][:, j * TW:(j + 1) * TW], o_sb[:])

    nc.compile()
    return nc


# ---------------------------------------------------------------------------
# FFA (fast FIR / polyphase Karatsuba) mode.
#
# out[n] = sum_t w[t] xp[n+t] splits by even/odd phases of n and t into 4
# quarter-size correlations; Karatsuba computes them with 3:
#   P0 = corr(X0, W0)        W0[s] = w[2s]
#   Q  = corr([0]+X1, W1h)   W1h[s] = w[2s-1] (leading zero)
#   PS = corr(X0+X1, W0+W1h)
#   out[2m] = P0[m] + Q[m];  out[2m+1] = PS[m] - P0[m] - Q[m+1]
# Applied recursively FFA_LVL times: 3^L correlations of (K>>L)+1 taps on
# 2^L-decimated signals -> (4/3)^L fewer PE MACs at full f16 precision.
# Signals stay phase-separated end to end: the host ships x de-interleaved
# into 2^L phases (layout only); the S-combos, recombination tree and the
# final interleave run on the vector/scalar engines under the matmuls.
# ---------------------------------------------------------------------------
FFA_LVL = int(os.environ.get("BASS_FFA_LVL", "4"))
NPH = 2 ** FFA_LVL
NLEAF = 3 ** FFA_LVL
LEAF_TAPS = (K >> FFA_LVL) + 1
PH_LEN = LP // NPH
LMARG = 8
PH_PAD = LMARG + PH_LEN + 24
RC = OUT_LEN // NPH              # valid cols per root output comp
LEAF_N = RC + FFA_LVL            # leaf matmul width (extra cols guard shifts)
CTW = LEAF_N + 4                 # out-comp tile width
WBUFS = 6


def _build_ffa_program():
    nc = bacc.Bacc("TRN2", target_bir_lowering=False, debug=False,
                   num_devices=N_CORES)
    x_d = nc.dram_tensor("x", [BPC, CIN, NPH * PH_PAD], F16,
                         kind="ExternalInput").ap()
    w_d = nc.dram_tensor("w", [NLEAF, CIN, LEAF_TAPS, COUT], F16,
                         kind="ExternalInput").ap()
    b_d = nc.dram_tensor("b", [COUT, 1], F32, kind="ExternalInput").ap()
    o_d = nc.dram_tensor("out", [BPC, COUT, OUT_LEN], F32,
                         kind="ExternalOutput").ap()

    with tile.TileContext(nc) as tc:
        with (
            tc.tile_pool(name="xp", bufs=1) as xpool,
            tc.tile_pool(name="wp", bufs=1) as wpool,
            tc.tile_pool(name="bp", bufs=1) as bpool,
            tc.tile_pool(name="oc", bufs=1) as ocpool,
            tc.tile_pool(name="of", bufs=3) as ofpool,
            tc.tile_pool(name="ps", bufs=8, space=bass.MemorySpace.PSUM) as ps,
        ):
            b_sb = bpool.tile([COUT, 1], F32)
            nc.sync.dma_start(b_sb[:], b_d[:])

            # --- X phases + S-combo tree (per batch) ---------------------
            # One wide tile per batch (phases side by side); chunked DMA so
            # the first leaves can start before the whole batch lands.
            x_all = {}
            XCH = 4
            for b in range(BPC):
                t = xpool.tile([CIN, NPH * PH_PAD], F16, tag=f"xa{b}",
                               name=f"xa{b}")
                cw = NPH * PH_PAD // XCH
                for c in range(XCH):
                    nc.sync.dma_start(t[:, c * cw:(c + 1) * cw],
                                      x_d[b][:, c * cw:(c + 1) * cw])
                x_all[b] = t

            def x_children(comps, b, lvl, path):
                """comps: list of (tile, base). Returns (c0, c1, cS)."""
                r = len(comps)
                c0 = comps[0::2]
                last_t, last_b = comps[-1]
                c1 = [(last_t, last_b - 1)] + comps[1:-1:2]
                codd = comps[1::2]
                cS = []
                for i in range(r // 2):
                    t0, o0 = c0[i]
                    t1, o1 = codd[i]
                    st = xpool.tile([CIN, PH_PAD], F16,
                                    tag=f"xs{b}_{path}_{i}",
                                    name=f"xs{b}_{path}_{i}")
                    wdt = PH_PAD - LMARG
                    nc.vector.tensor_add(
                        st[:, 4:4 + wdt],
                        t0[:, o0 - LMARG + 4:o0 - LMARG + 4 + wdt],
                        t1[:, o1 - LMARG + 4:o1 - LMARG + 4 + wdt])
                    cS.append((st, LMARG))
                return c0, c1, cS

            def build_x_leaves(b):
                leaves = []

                def rec(comps, lvl, path):
                    if lvl == FFA_LVL:
                        assert len(comps) == 1
                        leaves.append(comps[0])
                        return
                    c0, c1, cS = x_children(comps, b, lvl, path)
                    rec(c0, lvl + 1, path + "0")
                    rec(c1, lvl + 1, path + "1")
                    rec(cS, lvl + 1, path + "2")

                rec([(x_all[b], p * PH_PAD + LMARG) for p in range(NPH)],
                    0, "")
                return leaves

            x_leaves = {b: build_x_leaves(b) for b in range(BPC)}

            # --- output DFS: leaf matmuls + recombination tree -----------
            leaf_ctr = [0]

            def emit(lvl, slot):
                wdt = RC + lvl
                if lvl == FFA_LVL:
                    lc = leaf_ctr[0]
                    leaf_ctr[0] += 1
                    w_sb = wpool.tile([CIN, LEAF_TAPS, COUT], F16,
                                      tag=f"w{lc % WBUFS}",
                                      name=f"w{lc % WBUFS}")
                    nc.sync.dma_start(w_sb[:], w_d[lc])
                    res = {}
                    for b in range(BPC):
                        xt, base = x_leaves[b][lc]
                        acc = ps.tile([COUT, LEAF_N], F32)
                        for s in range(LEAF_TAPS):
                            nc.tensor.matmul(
                                acc[:], w_sb[:, s, :],
                                xt[:, base + s:base + s + LEAF_N],
                                start=(s == 0), stop=(s == LEAF_TAPS - 1))
                        d = ocpool.tile([COUT, CTW], F16,
                                        tag=f"oc{b}_{lvl}_{slot}_0",
                                        name=f"oc{b}_{lvl}_{slot}_0")
                        nc.scalar.activation(
                            d[:, :LEAF_N], acc[:],
                            mybir.ActivationFunctionType.Copy)
                        res[b] = [d]
                    return res

                ch0 = emit(lvl + 1, 0)
                ch1 = emit(lvl + 1, 1)
                chS = emit(lvl + 1, 2)
                r2 = 2 ** (FFA_LVL - lvl - 1)
                res = {}
                for b in range(BPC):
                    comps = []
                    for pp in range(r2):
                        # split root-level combos across DVE and GpSimd so
                        # the post-last-matmul chain is shorter
                        eng = nc.gpsimd if (lvl == 0 and pp % 2) else nc.vector
                        et = ocpool.tile([COUT, CTW], F16,
                                         tag=f"oc{b}_{lvl}_{slot}_e{pp}",
                                         name=f"oc{b}_{lvl}_{slot}_e{pp}")
                        eng.tensor_add(et[:, :wdt],
                                       ch0[b][pp][:, :wdt],
                                       ch1[b][pp][:, :wdt])
                        ot = ocpool.tile([COUT, CTW], F16,
                                         tag=f"oc{b}_{lvl}_{slot}_o{pp}",
                                         name=f"oc{b}_{lvl}_{slot}_o{pp}")
                        eng.tensor_sub(ot[:, :wdt],
                                       chS[b][pp][:, :wdt],
                                       ch0[b][pp][:, :wdt])
                        if pp + 1 < r2:
                            qs = ch1[b][pp + 1][:, 0:wdt]
                        else:
                            qs = ch1[b][0][:, 1:wdt + 1]
                        eng.tensor_sub(ot[:, :wdt], ot[:, :wdt], qs)
                        comps.append(et)
                        comps.append(ot)
                    res[b] = comps
                return res

            root = emit(0, 0)

            # --- final interleave + bias -> fp32, DMA out ----------------
            HC = OUT_LEN // 2            # 2016
            HR = RC // 2                 # 126 cols per comp per half
            for b in range(BPC):
                for h in range(2):
                    o32 = ofpool.tile([COUT, HC], F32)
                    for p in range(NPH):
                        dst = o32[:, p:p + (HR - 1) * NPH + 1:NPH]
                        src = root[b][p][:, h * HR:(h + 1) * HR]
                        if p % 2:
                            nc.vector.tensor_scalar_add(dst, src, b_sb[:])
                        else:
                            nc.scalar.activation(
                                dst, src,
                                mybir.ActivationFunctionType.Identity,
                                bias=b_sb[:], scale=1.0)
                    nc.sync.dma_start(o_d[b][:, h * HC:(h + 1) * HC], o32[:])

    nc.compile()
    return nc


def _build_program(mode: str, order: str):
    """mode: f32 | f32r | bf16 | f16 | f8 | ffa;  order: tap_inner | tap_outer."""
    if mode == "f8":
        return _build_f8_program()
    if mode == "ffa":
        return _build_ffa_program()
    io_dt = {"f32": F32, "f32r": F32R, "bf16": BF16, "f16": F16}[mode]
    nc = bacc.Bacc("TRN2", target_bir_lowering=False, debug=False,
                   num_devices=N_CORES)

    x_d = nc.dram_tensor("x", [BPC, CIN, LP], io_dt, kind="ExternalInput").ap()
    w_d = nc.dram_tensor("w", [CIN, K * COUT], io_dt, kind="ExternalInput").ap()
    b_d = nc.dram_tensor("b", [COUT, 1], F32, kind="ExternalInput").ap()
    o_d = nc.dram_tensor("out", [BPC, COUT, OUT_LEN], F32,
                         kind="ExternalOutput").ap()

    chunks = _tap_chunks()

    with tile.TileContext(nc) as tc:
        with (
            tc.tile_pool(name="wp", bufs=1) as wp,
            tc.tile_pool(name="xp", bufs=1) as xp,
            tc.tile_pool(name="bp", bufs=1) as bp,
            tc.tile_pool(name="op", bufs=4) as op,
            tc.tile_pool(name="ps", bufs=8 if order == "tap_inner" else 1,
                         space=bass.MemorySpace.PSUM) as ps,
        ):
            # DMA priority order: the first matmul group (batch 0, tile 0)
            # only needs x[0][:, :1136] and weight chunk 0, so issue those
            # first; the rest streams in behind while the PE is already busy.
            w_sb = [wp.tile([CIN, (t1 - t0) * COUT], io_dt, tag=f"w{ci}",
                            name=f"wsb{ci}")
                    for ci, (t0, t1) in enumerate(chunks)]
            x_sb = [xp.tile([CIN, LP], io_dt, tag=f"x{b}", name=f"xsb{b}")
                    for b in range(BPC)]

            # Critical path: tile (b=0, j=0) reads x[0][:, :632] and all taps.
            # Issue a small x slice, then weight chunks (smallest first), with
            # the rest of x interleaved behind the first two weight chunks.
            XA = TW + COUT      # columns needed by the first tile group
            XB = 2 * TW + COUT  # ... by the first two tile groups
            nc.sync.dma_start(x_sb[0][:, :XA], x_d[0][:, :XA])
            for ci, (t0, t1) in enumerate(chunks):
                nc.sync.dma_start(w_sb[ci][:], w_d[:, t0 * COUT:t1 * COUT])
                if ci == 0:
                    nc.sync.dma_start(x_sb[0][:, XA:XB], x_d[0][:, XA:XB])
                elif ci == 1:
                    nc.sync.dma_start(x_sb[0][:, XB:], x_d[0][:, XB:])
            b_sb = bp.tile([COUT, 1], F32)
            nc.sync.dma_start(b_sb[:], b_d[:])
            for b in range(1, BPC):
                nc.sync.dma_start(x_sb[b][:], x_d[b])

            def w_ap(t):
                for ci, (t0, t1) in enumerate(chunks):
                    if t0 <= t < t1:
                        return w_sb[ci][:, (t - t0) * COUT:(t - t0 + 1) * COUT]
                raise AssertionError

            def drain(psum_tile, b, j):
                o_sb = op.tile([COUT, TW], F32)
                nc.vector.tensor_scalar_add(o_sb[:], psum_tile[:], b_sb[:])
                nc.sync.dma_start(o_d[b][:, j * TW:(j + 1) * TW], o_sb[:])

            if order == "tap_inner":
                for b in range(BPC):
                    for j in range(NT):
                        acc = ps.tile([COUT, TW], F32)
                        for t in range(K):
                            nc.tensor.matmul(
                                acc[:],
                                w_ap(t),
                                x_sb[b][:, j * TW + t: j * TW + t + TW],
                                start=(t == 0), stop=(t == K - 1),
                            )
                        drain(acc, b, j)
            else:  # tap_outer
                for b in range(BPC):
                    accs = [ps.tile([COUT, TW], F32, tag=f"acc{j}", name=f"accs{j}")
                            for j in range(NT)]
                    for t in range(K):
                        for j in range(NT):
                            nc.tensor.matmul(
                                accs[j][:],
                                w_ap(t),
                                x_sb[b][:, j * TW + t: j * TW + t + TW],
                                start=(t == 0), stop=(t == K - 1),
                            )
                    for j in range(NT):
                        drain(accs[j], b, j)

    nc.compile()
    return nc


def _get_program(mode, order):
    key = (mode, order)
    if key not in _cache:
        _cache[key] = _build_program(mode, order)
    return _cache[key]


def _round_tf32(a: np.ndarray) -> np.ndarray:
    """Round fp32 to TF32 (10 mantissa bits), round-to-nearest-even."""
    u = np.ascontiguousarray(a, dtype=np.float32).view(np.uint32)
    r = (u + np.uint32(0xFFF) + ((u >> np.uint32(13)) & np.uint32(1))) \
        & np.uint32(0xFFFFE000)
    return r.view(np.float32)


def _ffa_w_leaves(weight):
    """Leaf filters of the FFA tree: (NLEAF, CIN, LEAF_TAPS, COUT) f16."""
    wt = np.ascontiguousarray(np.transpose(
        np.asarray(weight, np.float32), (2, 1, 0)))   # (K, CIN, COUT)
    nodes = [wt]
    for _ in range(FFA_LVL):
        nxt = []
        for a in nodes:
            z = np.zeros_like(a[:1])
            w0 = a[0::2]
            w1h = np.concatenate([z, a[1::2]], axis=0)
            nxt += [w0, w1h, w0 + w1h]
        nodes = nxt
    arr = np.stack(nodes)                              # (NLEAF, T, CIN, COUT)
    return np.ascontiguousarray(
        arr.transpose(0, 2, 1, 3).astype(np.float16))


def _ffa_x_phases(xp):
    """De-interleave xp into 2^L phases with zero margins (layout only).
    cin-major so each batch is one long-row DMA: (B, CIN, NPH*PH_PAD)."""
    ph = np.zeros((B, CIN, NPH, PH_PAD), np.float16)
    ph[:, :, :, LMARG:LMARG + PH_LEN] = \
        xp.reshape(B, CIN, PH_LEN, NPH).transpose(0, 1, 3, 2)
    return np.ascontiguousarray(ph.reshape(B, CIN, NPH * PH_PAD))


def kernel(x, weight, bias, _trace=False, _trace_kwargs=None):
    mode = os.environ.get("BASS_CONV_MODE", "ffa")
    order = os.environ.get("BASS_CONV_ORDER", "tap_inner")
    nc = _get_program(mode, order)

    if mode == "ffa":
        xp = np.zeros((B, CIN, LP), dtype=np.float32)
        xp[:, :, PAD:PAD + L] = x
        xph = _ffa_x_phases(xp)
        wl = _ffa_w_leaves(weight)
        b2 = np.ascontiguousarray(np.asarray(bias, np.float32)
                                  .reshape(COUT, 1))
        in_maps = [
            {"x": xph[c * BPC:(c + 1) * BPC], "w": wl, "b": b2}
            for c in range(N_CORES)
        ]
        res = run_bass_kernel_spmd(
            nc, in_maps, list(range(N_CORES)),
            trace=_trace, **(_trace_kwargs or {}),
        )
        out = np.concatenate(
            [res.results[c]["out"] for c in range(N_CORES)],
            axis=0).astype(np.float32)
        if _trace:
            return out, res
        return out

    if mode == "bf16":
        import ml_dtypes
        io_np = ml_dtypes.bfloat16
    elif mode == "f16":
        io_np = np.float16
    elif mode == "f8":
        import ml_dtypes
        io_np = ml_dtypes.float8_e4m3
    else:
        io_np = np.float32

    xp = np.zeros((B, CIN, LP), dtype=np.float32)
    xp[:, :, PAD:PAD + L] = x
    # (Cout, Cin, K) -> (Cin, K, Cout) so per-tap lhsT slices are contiguous
    wT = np.ascontiguousarray(np.transpose(
        np.asarray(weight, dtype=np.float32), (1, 2, 0)))
    if mode == "f32r":
        xp = _round_tf32(xp)
        wT = _round_tf32(wT)
    if mode == "f8":
        xp = np.clip(xp * SX, -240.0, 240.0)
        wT = np.clip(wT * SW, -240.0, 240.0)
    xp = np.ascontiguousarray(xp.astype(io_np))
    wT = np.ascontiguousarray(wT.astype(io_np))
    if mode != "f8":
        wT = wT.reshape(CIN, K * COUT)
    b2 = np.ascontiguousarray(np.asarray(bias, np.float32).reshape(COUT, 1))

    in_maps = [
        {"x": xp[c * BPC:(c + 1) * BPC], "w": wT, "b": b2}
        for c in range(N_CORES)
    ]
    res = run_bass_kernel_spmd(
        nc, in_maps, list(range(N_CORES)),
        trace=_trace, **(_trace_kwargs or {}),
    )
    out = np.concatenate([res.results[c]["out"] for c in range(N_CORES)],
                         axis=0).astype(np.float32)
    if _trace:
        return out, res
    return out

